# revision 3
# baseline (speedup 1.0000x reference)
"""Delta-accumulation GRU kernel for Trainium2 (8 NeuronCores, no
collectives; data-parallel over batch, 64 rows/core).

Gate pre-activations live in PSUM across all 64 steps:
    S_t = S_{t-1} + d_{t-1} @ W     where d = h_t - h_{t-1}
so steps 3..64 stream only delta matmuls (no bias rows).

v2 layout — split gates across the two PE quadrant streams: g0
(tile_position (0,0), PSUM parts 0-63) computes each gate's cols 0-511,
g1 ((0,64), parts 64-127) cols 512-1023.  Each gate gets one PSUM bank
as [128, 512] (part p, free c -> batch row p%64, gate col c+512*(p>=64)),
so the PSUM-reading tail ops (sigmoid(r), r*ghn, +gin) run at free-size
512 on 128 partitions — half the engine time of the batch-major v1.
The split->batch-major handoff happens inside ops with a PSUM input or
single-input Act ops (cross-partition-base reads are legal there; DVE
tensor-tensor with both inputs in SBUF requires equal bases, and mixing
T0/T8 transpose row-groups hangs this stack, so nn/vp/zc/dd/h are
batch-major [64, 1024] and all 8 transposes are T0).

Per-stream slot schedule (40 slots): r+ghn interleaved (slots 1-16, all
four a-half k's first, so the b-half k-tiles {2,3,6,7} are first consumed
at slot 9), then gin, z-a (256 cols), z-b — completions staggered at
40/40/60/80/100% of the phase.  Per-chunk k consumption order KSEQ
matches production order (z-a produces k{0,1,4,5}, z-b k{2,3,6,7}).
The a-half transposes are injected 4 slots before phase end; the b-half
transposes + xT copies + h update are deferred into the NEXT phase after
its first 4 slots, so the PE never head-of-line blocks on the tail.

Verified on HW: rel_err 0.00856 vs the fp32 reference (gate 2e-2);
~8.0-8.5us/step steady (v1 baseline: ~10-13us/step same protocol).
"""

import numpy as np
import ml_dtypes

import concourse.bass as bass
import concourse.bacc as bacc
import concourse.mybir as mybir
import concourse.tile as tile
from concourse.bass_utils import run_bass_kernel_spmd
from concourse.masks import make_identity

BF16 = mybir.dt.bfloat16
F32 = mybir.dt.float32
AF = mybir.ActivationFunctionType

B, D, T = 512, 1024, 64
NCORES = 8
BL = B // NCORES
CTX = 3072
NK = D // 128
NKC = CTX // 128
CH = 512
QW = 256

KSEQ = [0, 4, 1, 5, 2, 6, 3, 7]
INJ_B = 4
INJ_A_FROM_END = 4

_CACHE = {}
TRACE = False
TRACE_KW = {}
LAST_RESULT = [None]
LAST_IN_MAPS = [None]


def _build_nc(n_steps=T, mode="normal"):
    nc = bacc.Bacc("TRN2")

    ctxT_h = nc.declare_dram_parameter("ctxT", [CTX, BL], BF16, isOutput=False)
    wctx_h = nc.declare_dram_parameter("wctx", [CTX, D], BF16, isOutput=False)
    whh_h = nc.declare_dram_parameter("whh", [D, 3 * D], BF16, isOutput=False)
    wall_h = nc.declare_dram_parameter("wall", [D, 4 * D], BF16, isOutput=False)
    bctx_h = nc.declare_dram_parameter("bctx", [1, D], BF16, isOutput=False)
    bias1_h = nc.declare_dram_parameter("bias1", [1, 3 * D], BF16, isOutput=False)
    gin1_h = nc.declare_dram_parameter("gin1", [1, D], F32, isOutput=False)
    biasM_h = nc.declare_dram_parameter("biasM", [1, 4 * D], BF16, isOutput=False)
    out_h = nc.declare_dram_parameter("out", [T, BL, D], F32, isOutput=True)

    with tile.TileContext(nc) as tc:
        with (
            tc.tile_pool(name="wres", bufs=1) as wres,
            tc.tile_pool(name="wstream", bufs=4) as wstream,
            tc.tile_pool(name="consts", bufs=1) as consts,
            tc.tile_pool(name="state", bufs=2) as state,
            tc.tile_pool(name="work", bufs=1) as work,
            tc.tile_pool(name="psum", bufs=1, space="PSUM") as psum,
        ):
            ctxT_sb = consts.tile([128, NKC, BL], BF16)
            nc.sync.dma_start(
                out=ctxT_sb, in_=ctxT_h[:].rearrange("(ko p) b -> p ko b", p=128)
            )
            whh_sb = wres.tile([128, NK, 3 * D], BF16, tag="whh")
            whh_t = whh_h[:].rearrange("(ko p) n -> p ko n", p=128)
            for q in range(4):
                nc.sync.dma_start(
                    out=whh_sb[:, 2 * q : 2 * q + 2, :],
                    in_=whh_t[:, 2 * q : 2 * q + 2, :],
                )
            wall_sb = wres.tile([128, NK, 4 * D], BF16, tag="wall")
            wall_t = wall_h[:].rearrange("(ko p) n -> p ko n", p=128)
            for q in range(4):
                nc.sync.dma_start(
                    out=wall_sb[:, 2 * q : 2 * q + 2, :],
                    in_=wall_t[:, 2 * q : 2 * q + 2, :],
                )
            wctx_t = wctx_h[:].rearrange("(ko p) n -> p ko n", p=128)
            bctx_sb = consts.tile([1, D], BF16)
            nc.sync.dma_start(out=bctx_sb, in_=bctx_h[:])
            bias1_sb = consts.tile([1, 3 * D], BF16)
            nc.sync.dma_start(out=bias1_sb, in_=bias1_h[:])
            biasM_sb = consts.tile([1, 4 * D], BF16)
            nc.sync.dma_start(out=biasM_sb, in_=biasM_h[:])
            # step-1 n-gate input term, broadcast into split shape
            gin1_bc = consts.tile([128, CH], F32)
            g1ap = gin1_h[:]
            lo = bass.AP(tensor=g1ap.tensor, offset=g1ap.offset,
                         ap=[[0, BL], [1, CH]])
            hi = bass.AP(tensor=g1ap.tensor, offset=g1ap.offset + CH,
                         ap=[[0, BL], [1, CH]])
            nc.gpsimd.dma_start(out=gin1_bc[0:64, :], in_=lo)
            nc.gpsimd.dma_start(out=gin1_bc[64:128, :], in_=hi)
            ones_sb = consts.tile([1, BL], BF16)
            nc.vector.memset(ones_sb, 1.0)
            ident_bf = consts.tile([BL, BL], BF16)
            make_identity(nc, ident_bf)

            # PSUM: one bank per gate chunk + transpose stage + h0
            pR = psum.tile([128, CH], F32, tag="pR")
            pZa = psum.tile([128, CH], F32, tag="pZa")
            pZb = psum.tile([128, CH], F32, tag="pZb")
            pG = psum.tile([128, CH], F32, tag="pG")
            pH = psum.tile([128, CH], F32, tag="pH")
            pS = psum.tile([128, CH], F32, tag="pS")
            pI = psum.tile([128, CH], F32, tag="pI")
            stage = (pS[:].bitcast(BF16)[:, 0 : NK * BL]
                     .rearrange("p (j c) -> p j c", c=BL))
            stage4 = stage.rearrange("p (h j) c -> p h j c", h=2)

            TPS = [(0, 0), (0, 64)]

            def chunks_for(base_r, base_z, base_gin, base_ghn, with_gin):
                cks = []
                for p0, p1, off in [(0, 64, 0), (64, 128, CH)]:
                    cs = [(pR[p0:p1, :], base_r + off, CH),
                          (pH[p0:p1, :], base_ghn + off, CH)]
                    if with_gin:
                        cs.append((pG[p0:p1, :], base_gin + off, CH))
                    cs.append((pZa[p0:p1, 0:QW], base_z + off, QW))
                    cs.append((pZb[p0:p1, 0:QW], base_z + off + QW, QW))
                    cks.append(cs)
                return cks

            CK_STEADY = chunks_for(0, D, 2 * D, 3 * D, with_gin=True)
            CK_STEP1 = chunks_for(0, D, None, 2 * D, with_gin=False)

            def transpose_j(src, j):
                """src: batch-major [64, D] bf16; k-tile j -> stage[:, j, :]"""
                nc.tensor.transpose(
                    stage[:, j, :], src[:, 128 * j : 128 * (j + 1)], ident_bf
                )

            def make_xT(i):
                return state.tile([128, NK, BL], BF16, tag="xT", bufs=2,
                                  name=f"xT_{i}")

            def emit_step(s, hprev, hprev_bf, xT_in, step1, full, last, pend,
                          w_sb, bias_sb, cks, tail_on):
                i_id = nc.next_id()
                nslots = len(cks[0]) * NK
                za_end = nslots - NK
                inj_a = nslots - INJ_A_FROM_END
                if full:
                    for st in (0, 1):
                        for pap, wcol, width in cks[st]:
                            nc.tensor.matmul(
                                pap, ones_sb[0:1, :],
                                bias_sb[0:1, wcol : wcol + width],
                                start=True, stop=False, tile_position=TPS[st],
                                skip_group_check=True,
                            )

                def slot_order(cs):
                    head = [(c, k) for k in KSEQ for c in cs[:2]]
                    tail_s = [(c, k) for c in cs[2:] for k in KSEQ]
                    return head + tail_s

                slots = [slot_order(cks[st]) for st in (0, 1)]

                rs = work.tile([128, CH], BF16, tag="rs", name=f"rs_{i_id}")
                tt = work.tile([128, CH], F32, tag="tt", name=f"tt_{i_id}")
                uu = work.tile([128, CH], F32, tag="uu", name=f"uu_{i_id}")
                nn = work.tile([BL, D], BF16, tag="nn", name=f"nn_{i_id}")
                vp = work.tile([BL, D], BF16, tag="vp", name=f"vp_{i_id}")
                zc = work.tile([BL, D], BF16, tag="zc", name=f"zc_{i_id}")
                dd = work.tile([BL, D], BF16, tag="dd", bufs=2,
                               name=f"dd_{i_id}")
                hnew = state.tile([BL, D], F32, bufs=3, tag="h",
                                  name=f"h_{i_id}")
                hbf = state.tile([BL, D], BF16, bufs=3, tag="hbf",
                                 name=f"hbf_{i_id}")
                nn2 = nn.rearrange("b (h c) -> b h c", h=2)
                vp2 = vp.rearrange("b (h c) -> b h c", h=2)
                zc2 = zc.rearrange("b (h c) -> b h c", h=2)
                dd2 = dd.rearrange("b (h c) -> b h c", h=2)
                hb2 = hprev_bf.rearrange("b (h c) -> b h c", h=2)
                xT = make_xT(i_id) if (tail_on and not last) else None
                x4 = (xT.rearrange("p (h j) c -> p h j c", h=2)
                      if xT is not None else None)
                gin_src = gin1_bc if step1 else pG

                def a_chain():
                    nc.scalar.activation(rs, pR, AF.Sigmoid)
                    nc.vector.tensor_mul(tt, rs, pH)
                    nc.vector.tensor_add(uu[:, 0:QW], tt[:, 0:QW],
                                         gin_src[:, 0:QW])
                    nc.vector.tensor_add(uu[:, QW:CH], tt[:, QW:CH],
                                         gin_src[:, QW:CH])
                    # split -> batch-major handoff: Act single-input reads
                    # cross partition bases; zc reads PSUM cross-base
                    nc.scalar.activation(nn[:, 0:QW], uu[0:64, 0:QW], AF.Tanh)
                    nc.scalar.activation(nn[:, CH : CH + QW], uu[64:128, 0:QW],
                                         AF.Tanh)
                    nc.vector.tensor_sub(vp2[:, :, 0:QW], nn2[:, :, 0:QW],
                                         hb2[:, :, 0:QW])
                    nc.scalar.activation(zc[:, 0:QW], pZa[0:64, 0:QW],
                                         AF.Sigmoid, scale=-1.0)
                    nc.scalar.activation(zc[:, CH : CH + QW], pZa[64:128, 0:QW],
                                         AF.Sigmoid, scale=-1.0)
                    nc.vector.tensor_mul(dd2[:, :, 0:QW], zc2[:, :, 0:QW],
                                         vp2[:, :, 0:QW])

                for i in range(nslots):
                    for st in (0, 1):
                        (pap, wcol, width), k = slots[st][i]
                        nc.tensor.matmul(
                            pap, xT_in[:, k, :],
                            w_sb[:, k, wcol : wcol + width],
                            start=False, stop=(k == KSEQ[-1]),
                            tile_position=TPS[st], skip_group_check=True,
                        )
                    if i + 1 == INJ_B and pend is not None:
                        pend()
                    if tail_on and i + 1 == za_end:
                        a_chain()
                    if tail_on and not last and not step1 and i + 1 == inj_a:
                        for j in (0, 1, 4, 5):
                            transpose_j(dd, j)

                if not tail_on:
                    return hprev, hprev_bf, xT_in, None
                # a-half copies: (k0,k4) then (k1,k5)
                if not last and not step1:
                    nc.vector.tensor_copy(x4[:, :, 0:1, :], stage4[:, :, 0:1, :])
                    nc.vector.tensor_copy(x4[:, :, 1:2, :], stage4[:, :, 1:2, :])
                # b-half chain
                nc.scalar.activation(nn[:, QW:CH], uu[0:64, QW:CH], AF.Tanh)
                nc.scalar.activation(nn[:, CH + QW : D], uu[64:128, QW:CH],
                                     AF.Tanh)
                nc.vector.tensor_sub(vp2[:, :, QW:CH], nn2[:, :, QW:CH],
                                     hb2[:, :, QW:CH])
                nc.scalar.activation(zc[:, QW:CH], pZb[0:64, 0:QW], AF.Sigmoid,
                                     scale=-1.0)
                nc.scalar.activation(zc[:, CH + QW : D], pZb[64:128, 0:QW],
                                     AF.Sigmoid, scale=-1.0)
                nc.vector.tensor_mul(dd2[:, :, QW:CH], zc2[:, :, QW:CH],
                                     vp2[:, :, QW:CH])
                if last:
                    nc.vector.tensor_add(hnew, hprev, dd)
                    nc.sync.dma_start(out=out_h[s], in_=hnew)
                    return hnew, None, None, None

                if step1:
                    # step 2 is a full write with x == h1: transpose h1, not d1
                    nc.vector.tensor_add(hnew, hprev, dd)
                    nc.scalar.copy(hbf, hnew)
                    for j in range(NK):
                        transpose_j(hbf, j)
                    nc.vector.tensor_copy(xT, stage)
                    nc.sync.dma_start(out=out_h[s], in_=hnew)
                    return hnew, hbf, xT, None

                def pend_next():
                    # b-half transposes + copies + h update, injected into
                    # the next phase after its first INJ_B slots
                    for j in (2, 3, 6, 7):
                        transpose_j(dd, j)
                    nc.vector.tensor_copy(x4[:, :, 2:3, :], stage4[:, :, 2:3, :])
                    nc.vector.tensor_copy(x4[:, :, 3:4, :], stage4[:, :, 3:4, :])
                    nc.vector.tensor_add(hnew, hprev, dd)
                    nc.gpsimd.tensor_copy(hbf, hnew)
                    nc.sync.dma_start(out=out_h[s], in_=hnew)

                return hnew, hbf, xT, pend_next

            # ---- h0 = ctx @ W_ctx + bctx (split across streams) ----
            for st, (p0, p1, off) in enumerate([(0, 64, 0), (64, 128, CH)]):
                nc.tensor.matmul(
                    pI[p0:p1, :], ones_sb[0:1, :], bctx_sb[0:1, off : off + CH],
                    start=True, stop=False, tile_position=TPS[st],
                    skip_group_check=True,
                )
            for kc in range(NKC):
                wk = wstream.tile([128, D], BF16, tag="wctxk", name=f"wk_{kc}")
                nc.gpsimd.dma_start(out=wk, in_=wctx_t[:, kc, :])
                for st, (p0, p1, off) in enumerate([(0, 64, 0), (64, 128, CH)]):
                    nc.tensor.matmul(
                        pI[p0:p1, :], ctxT_sb[:, kc, :], wk[:, off : off + CH],
                        start=False, stop=(kc == NKC - 1),
                        tile_position=TPS[st], skip_group_check=True,
                    )
            h0_sb = state.tile([BL, D], F32, tag="h", bufs=3)
            h0_bf = state.tile([BL, D], BF16, bufs=3, tag="hbf")
            nc.vector.tensor_copy(h0_sb[:, 0:CH], pI[0:64, :])
            nc.vector.tensor_copy(h0_sb[:, CH:D], pI[64:128, :])
            nc.scalar.copy(h0_bf, h0_sb)
            for j in range(NK):
                transpose_j(h0_bf, j)
            xT0 = make_xT(0)
            nc.vector.tensor_copy(xT0, stage)

            tail_on = mode != "mm_only"
            hprev, hprev_bf, xT_cur, pend = emit_step(
                0, h0_sb, h0_bf, xT0, step1=True, full=True,
                last=(n_steps == 1), pend=None, w_sb=whh_sb,
                bias_sb=bias1_sb, cks=CK_STEP1, tail_on=True,
            )
            if n_steps >= 2:
                hprev, hprev_bf, xT_cur, pend = emit_step(
                    1, hprev, hprev_bf, xT_cur, step1=False, full=True,
                    last=(n_steps == 2), pend=pend, w_sb=wall_sb,
                    bias_sb=biasM_sb, cks=CK_STEADY, tail_on=True,
                )
            for s in range(2, n_steps):
                hprev, hprev_bf, xT_cur, pend = emit_step(
                    s % T, hprev, hprev_bf, xT_cur, step1=False, full=False,
                    last=(s == n_steps - 1), pend=pend, w_sb=wall_sb,
                    bias_sb=biasM_sb, cks=CK_STEADY, tail_on=tail_on,
                )

    nc.finalize()
    return nc


def kernel(world_state, goal, W_ctx, b_ctx, start_token, W_ih, b_ih, W_hh, b_hh):
    bf16 = ml_dtypes.bfloat16
    ws = np.asarray(world_state, dtype=np.float32)
    gl = np.asarray(goal, dtype=np.float32)
    W_ctx = np.asarray(W_ctx, dtype=np.float32)
    b_ctx = np.asarray(b_ctx, dtype=np.float32)
    start_token = np.asarray(start_token, dtype=np.float32)
    W_ih = np.asarray(W_ih, dtype=np.float32)
    b_ih = np.asarray(b_ih, dtype=np.float32)
    W_hh = np.asarray(W_hh, dtype=np.float32)
    b_hh = np.asarray(b_hh, dtype=np.float32)

    if "nc" not in _CACHE:
        _CACHE["nc"] = _build_nc()
    nc = _CACHE["nc"]

    ctxT = np.ascontiguousarray(np.concatenate([ws, gl], axis=1).T)
    ctxT_bf = ctxT.astype(bf16)
    wctx_bf = np.ascontiguousarray(W_ctx).astype(bf16)
    whh_bf = np.ascontiguousarray(W_hh).astype(bf16)
    wall_bf = np.ascontiguousarray(
        np.concatenate(
            [W_ih[:, : 2 * D] + W_hh[:, : 2 * D], W_ih[:, 2 * D :], W_hh[:, 2 * D :]],
            axis=1,
        )
    ).astype(bf16)
    gi1 = start_token @ W_ih + b_ih
    bias1 = np.ascontiguousarray(
        np.concatenate([gi1[: 2 * D] + b_hh[: 2 * D], b_hh[2 * D :]])
    ).astype(bf16)[None]
    gin1 = np.ascontiguousarray(gi1[2 * D :].astype(np.float32))[None]
    biasM = np.ascontiguousarray(
        np.concatenate([b_ih[: 2 * D] + b_hh[: 2 * D], b_ih[2 * D :], b_hh[2 * D :]])
    ).astype(bf16)[None]
    bctx = np.ascontiguousarray(b_ctx).astype(bf16)[None]

    shared = dict(
        wctx=wctx_bf, whh=whh_bf, wall=wall_bf, bctx=bctx,
        bias1=bias1, gin1=gin1, biasM=biasM,
    )
    in_maps = [
        {**shared, "ctxT": np.ascontiguousarray(ctxT_bf[:, i * BL : (i + 1) * BL])}
        for i in range(NCORES)
    ]

    LAST_IN_MAPS[0] = in_maps
    res = run_bass_kernel_spmd(
        nc, in_maps, core_ids=list(range(NCORES)), trace=TRACE, **TRACE_KW
    )
    LAST_RESULT[0] = res

    full = np.empty((B, T, D), dtype=np.float32)
    for i in range(NCORES):
        o = np.asarray(res.results[i]["out"])
        full[i * BL : (i + 1) * BL] = o.transpose(1, 0, 2)
    return full


# revision 4
# speedup vs baseline: 1.0815x; 1.0815x over previous
"""Delta-accumulation GRU kernel for Trainium2 (8 NeuronCores, no
collectives; data-parallel over batch, 64 rows/core).

Gate pre-activations live in PSUM across all 64 steps:
    S_t = S_{t-1} + d_{t-1} @ W     where d = h_t - h_{t-1}
so steps 3..64 stream only delta matmuls (no bias rows).

v2 layout — split gates across the two PE quadrant streams: g0
(tile_position (0,0), PSUM parts 0-63) computes each gate's cols 0-511,
g1 ((0,64), parts 64-127) cols 512-1023.  Each gate gets one PSUM bank
as [128, 512] (part p, free c -> batch row p%64, gate col c+512*(p>=64)),
so the PSUM-reading tail ops (sigmoid(r), r*ghn, +gin) run at free-size
512 on 128 partitions — half the engine time of the batch-major v1.
The split->batch-major handoff happens inside ops with a PSUM input or
single-input Act ops (cross-partition-base reads are legal there; DVE
tensor-tensor with both inputs in SBUF requires equal bases, and mixing
T0/T8 transpose row-groups hangs this stack, so nn/vp/zc/dd/h are
batch-major [64, 1024] and all 8 transposes are T0).

Per-stream slot schedule (40 slots): r+ghn interleaved (slots 1-16, all
four a-half k's first, so the b-half k-tiles {2,3,6,7} are first consumed
at slot 9), then gin, z-a (256 cols), z-b — completions staggered at
40/40/60/80/100% of the phase.  Per-chunk k consumption order KSEQ
matches production order (z-a produces k{0,1,4,5}, z-b k{2,3,6,7}).
The a-half transposes are injected 4 slots before phase end; the b-half
transposes + xT copies + h update are deferred into the NEXT phase after
its first 4 slots, so the PE never head-of-line blocks on the tail.

Verified on HW: rel_err 0.00856 vs the fp32 reference (gate 2e-2);
~8.0-8.5us/step steady (v1 baseline: ~10-13us/step same protocol).
"""

import numpy as np
import ml_dtypes

import concourse.bass as bass
import concourse.bacc as bacc
import concourse.mybir as mybir
import concourse.tile as tile
from concourse.bass_utils import run_bass_kernel_spmd
from concourse.masks import make_identity

BF16 = mybir.dt.bfloat16
F32 = mybir.dt.float32
AF = mybir.ActivationFunctionType

B, D, T = 512, 1024, 64
NCORES = 8
BL = B // NCORES
CTX = 3072
NK = D // 128
NKC = CTX // 128
CH = 512
QW = 256

KSEQ = [0, 4, 1, 5, 2, 6, 3, 7]
INJ_B = 4
INJ_A_FROM_END = 2

_CACHE = {}
TRACE = False
TRACE_KW = {}
LAST_RESULT = [None]
LAST_IN_MAPS = [None]


def _build_nc(n_steps=T, mode="normal"):
    nc = bacc.Bacc("TRN2")

    ctxT_h = nc.declare_dram_parameter("ctxT", [CTX, BL], BF16, isOutput=False)
    wctx_h = nc.declare_dram_parameter("wctx", [CTX, D], BF16, isOutput=False)
    whh_h = nc.declare_dram_parameter("whh", [D, 3 * D], BF16, isOutput=False)
    wall_h = nc.declare_dram_parameter("wall", [D, 4 * D], BF16, isOutput=False)
    bctx_h = nc.declare_dram_parameter("bctx", [1, D], BF16, isOutput=False)
    bias1_h = nc.declare_dram_parameter("bias1", [1, 3 * D], BF16, isOutput=False)
    gin1_h = nc.declare_dram_parameter("gin1", [1, D], F32, isOutput=False)
    biasM_h = nc.declare_dram_parameter("biasM", [1, 4 * D], BF16, isOutput=False)
    out_h = nc.declare_dram_parameter("out", [T, BL, D], F32, isOutput=True)

    with tile.TileContext(nc) as tc:
        with (
            tc.tile_pool(name="wres", bufs=1) as wres,
            tc.tile_pool(name="wstream", bufs=4) as wstream,
            tc.tile_pool(name="consts", bufs=1) as consts,
            tc.tile_pool(name="state", bufs=2) as state,
            tc.tile_pool(name="work", bufs=1) as work,
            tc.tile_pool(name="psum", bufs=1, space="PSUM") as psum,
        ):
            ctxT_sb = consts.tile([128, NKC, BL], BF16)
            nc.sync.dma_start(
                out=ctxT_sb, in_=ctxT_h[:].rearrange("(ko p) b -> p ko b", p=128)
            )
            whh_sb = wres.tile([128, NK, 3 * D], BF16, tag="whh")
            whh_t = whh_h[:].rearrange("(ko p) n -> p ko n", p=128)
            for q in range(4):
                nc.sync.dma_start(
                    out=whh_sb[:, 2 * q : 2 * q + 2, :],
                    in_=whh_t[:, 2 * q : 2 * q + 2, :],
                )
            wall_sb = wres.tile([128, NK, 4 * D], BF16, tag="wall")
            wall_t = wall_h[:].rearrange("(ko p) n -> p ko n", p=128)
            for q in range(4):
                nc.sync.dma_start(
                    out=wall_sb[:, 2 * q : 2 * q + 2, :],
                    in_=wall_t[:, 2 * q : 2 * q + 2, :],
                )
            wctx_t = wctx_h[:].rearrange("(ko p) n -> p ko n", p=128)
            bctx_sb = consts.tile([1, D], BF16)
            nc.sync.dma_start(out=bctx_sb, in_=bctx_h[:])
            bias1_sb = consts.tile([1, 3 * D], BF16)
            nc.sync.dma_start(out=bias1_sb, in_=bias1_h[:])
            biasM_sb = consts.tile([1, 4 * D], BF16)
            nc.sync.dma_start(out=biasM_sb, in_=biasM_h[:])
            # step-1 n-gate input term, broadcast into split shape
            gin1_bc = consts.tile([128, CH], F32)
            g1ap = gin1_h[:]
            lo = bass.AP(tensor=g1ap.tensor, offset=g1ap.offset,
                         ap=[[0, BL], [1, CH]])
            hi = bass.AP(tensor=g1ap.tensor, offset=g1ap.offset + CH,
                         ap=[[0, BL], [1, CH]])
            nc.gpsimd.dma_start(out=gin1_bc[0:64, :], in_=lo)
            nc.gpsimd.dma_start(out=gin1_bc[64:128, :], in_=hi)
            ones_sb = consts.tile([1, BL], BF16)
            nc.vector.memset(ones_sb, 1.0)
            ident_bf = consts.tile([BL, BL], BF16)
            make_identity(nc, ident_bf)

            # PSUM: one bank per gate chunk + transpose stage + h0
            pR = psum.tile([128, CH], F32, tag="pR")
            pZa = psum.tile([128, CH], F32, tag="pZa")
            pZb = psum.tile([128, CH], F32, tag="pZb")
            pG = psum.tile([128, CH], F32, tag="pG")
            pH = psum.tile([128, CH], F32, tag="pH")
            pS = psum.tile([128, CH], F32, tag="pS")
            pI = psum.tile([128, CH], F32, tag="pI")
            stage = (pS[:].bitcast(BF16)[:, 0 : NK * BL]
                     .rearrange("p (j c) -> p j c", c=BL))
            stage4 = stage.rearrange("p (h j) c -> p h j c", h=2)

            TPS = [(0, 0), (0, 64)]

            def chunks_for(base_r, base_z, base_gin, base_ghn, with_gin):
                cks = []
                for p0, p1, off in [(0, 64, 0), (64, 128, CH)]:
                    cs = [(pR[p0:p1, :], base_r + off, CH),
                          (pH[p0:p1, :], base_ghn + off, CH)]
                    if with_gin:
                        cs.append((pG[p0:p1, :], base_gin + off, CH))
                    cs.append((pZa[p0:p1, 0:QW], base_z + off, QW))
                    cs.append((pZb[p0:p1, 0:QW], base_z + off + QW, QW))
                    cks.append(cs)
                return cks

            CK_STEADY = chunks_for(0, D, 2 * D, 3 * D, with_gin=True)
            CK_STEP1 = chunks_for(0, D, None, 2 * D, with_gin=False)

            def transpose_j(src, j):
                """src: batch-major [64, D] bf16; k-tile j -> stage[:, j, :]"""
                nc.tensor.transpose(
                    stage[:, j, :], src[:, 128 * j : 128 * (j + 1)], ident_bf
                )

            def make_xT(i):
                return state.tile([128, NK, BL], BF16, tag="xT", bufs=2,
                                  name=f"xT_{i}")

            def emit_step(s, hprev, hprev_bf, xT_in, step1, full, last, pend,
                          w_sb, bias_sb, cks, tail_on):
                i_id = nc.next_id()
                nslots = len(cks[0]) * NK
                za_end = nslots - NK
                inj_a = nslots - INJ_A_FROM_END
                if full:
                    for st in (0, 1):
                        for pap, wcol, width in cks[st]:
                            nc.tensor.matmul(
                                pap, ones_sb[0:1, :],
                                bias_sb[0:1, wcol : wcol + width],
                                start=True, stop=False, tile_position=TPS[st],
                                skip_group_check=True,
                            )

                def slot_order(cs):
                    head = [(c, k) for k in KSEQ for c in cs[:2]]
                    tail_s = [(c, k) for c in cs[2:] for k in KSEQ]
                    return head + tail_s

                slots = [slot_order(cks[st]) for st in (0, 1)]

                rs = work.tile([128, CH], BF16, tag="rs", name=f"rs_{i_id}")
                tt = work.tile([128, CH], F32, tag="tt", name=f"tt_{i_id}")
                uu = work.tile([128, CH], F32, tag="uu", name=f"uu_{i_id}")
                nn = work.tile([BL, D], BF16, tag="nn", name=f"nn_{i_id}")
                vp = work.tile([BL, D], BF16, tag="vp", name=f"vp_{i_id}")
                zc = work.tile([BL, D], BF16, tag="zc", name=f"zc_{i_id}")
                dd = work.tile([BL, D], BF16, tag="dd", bufs=2,
                               name=f"dd_{i_id}")
                hnew = state.tile([BL, D], F32, bufs=3, tag="h",
                                  name=f"h_{i_id}")
                hbf = state.tile([BL, D], BF16, bufs=3, tag="hbf",
                                 name=f"hbf_{i_id}")
                nn2 = nn.rearrange("b (h c) -> b h c", h=2)
                vp2 = vp.rearrange("b (h c) -> b h c", h=2)
                zc2 = zc.rearrange("b (h c) -> b h c", h=2)
                dd2 = dd.rearrange("b (h c) -> b h c", h=2)
                hb2 = hprev_bf.rearrange("b (h c) -> b h c", h=2)
                xT = make_xT(i_id) if (tail_on and not last) else None
                x4 = (xT.rearrange("p (h j) c -> p h j c", h=2)
                      if xT is not None else None)
                gin_src = gin1_bc if step1 else pG

                def a_chain():
                    nc.scalar.activation(rs, pR, AF.Sigmoid)
                    nc.vector.tensor_mul(tt, rs, pH)
                    nc.vector.tensor_add(uu[:, 0:QW], tt[:, 0:QW],
                                         gin_src[:, 0:QW])
                    nc.vector.tensor_add(uu[:, QW:CH], tt[:, QW:CH],
                                         gin_src[:, QW:CH])
                    # split -> batch-major handoff: Act single-input reads
                    # cross partition bases; zc reads PSUM cross-base
                    nc.scalar.activation(nn[:, 0:QW], uu[0:64, 0:QW], AF.Tanh)
                    nc.scalar.activation(nn[:, CH : CH + QW], uu[64:128, 0:QW],
                                         AF.Tanh)
                    nc.vector.tensor_sub(vp2[:, :, 0:QW], nn2[:, :, 0:QW],
                                         hb2[:, :, 0:QW])
                    nc.scalar.activation(zc[:, 0:QW], pZa[0:64, 0:QW],
                                         AF.Sigmoid, scale=-1.0)
                    nc.scalar.activation(zc[:, CH : CH + QW], pZa[64:128, 0:QW],
                                         AF.Sigmoid, scale=-1.0)
                    nc.vector.tensor_mul(dd2[:, :, 0:QW], zc2[:, :, 0:QW],
                                         vp2[:, :, 0:QW])

                for i in range(nslots):
                    for st in (0, 1):
                        (pap, wcol, width), k = slots[st][i]
                        nc.tensor.matmul(
                            pap, xT_in[:, k, :],
                            w_sb[:, k, wcol : wcol + width],
                            start=False, stop=(k == KSEQ[-1]),
                            tile_position=TPS[st], skip_group_check=True,
                        )
                    if i + 1 == INJ_B and pend is not None:
                        pend()
                    if tail_on and i + 1 == za_end:
                        a_chain()
                    if tail_on and not last and not step1 and i + 1 == inj_a:
                        for j in (0, 4, 1, 5):
                            transpose_j(dd, j)

                if not tail_on:
                    return hprev, hprev_bf, xT_in, None
                # a-half copies, per k-tile in consumption order so slot 1
                # only waits on k0's copy
                if not last and not step1:
                    for j in (0, 4, 1, 5):
                        nc.vector.tensor_copy(xT[:, j, :], stage[:, j, :])
                # b-half chain
                nc.scalar.activation(nn[:, QW:CH], uu[0:64, QW:CH], AF.Tanh)
                nc.scalar.activation(nn[:, CH + QW : D], uu[64:128, QW:CH],
                                     AF.Tanh)
                nc.vector.tensor_sub(vp2[:, :, QW:CH], nn2[:, :, QW:CH],
                                     hb2[:, :, QW:CH])
                nc.scalar.activation(zc[:, QW:CH], pZb[0:64, 0:QW], AF.Sigmoid,
                                     scale=-1.0)
                nc.scalar.activation(zc[:, CH + QW : D], pZb[64:128, 0:QW],
                                     AF.Sigmoid, scale=-1.0)
                nc.vector.tensor_mul(dd2[:, :, QW:CH], zc2[:, :, QW:CH],
                                     vp2[:, :, QW:CH])
                if last:
                    nc.vector.tensor_add(hnew, hprev, dd)
                    nc.sync.dma_start(out=out_h[s], in_=hnew)
                    return hnew, None, None, None

                if step1:
                    # step 2 is a full write with x == h1: transpose h1, not d1
                    nc.vector.tensor_add(hnew, hprev, dd)
                    nc.scalar.copy(hbf, hnew)
                    for j in range(NK):
                        transpose_j(hbf, j)
                    nc.vector.tensor_copy(xT, stage)
                    nc.sync.dma_start(out=out_h[s], in_=hnew)
                    return hnew, hbf, xT, None

                def pend_next():
                    # b-half transposes + copies + h update, injected into
                    # the next phase after its first INJ_B slots
                    for j in (2, 3, 6, 7):
                        transpose_j(dd, j)
                    nc.vector.tensor_copy(x4[:, :, 2:3, :], stage4[:, :, 2:3, :])
                    nc.vector.tensor_copy(x4[:, :, 3:4, :], stage4[:, :, 3:4, :])
                    nc.vector.tensor_add(hnew, hprev, dd)
                    nc.gpsimd.tensor_copy(hbf, hnew)
                    nc.sync.dma_start(out=out_h[s], in_=hnew)

                return hnew, hbf, xT, pend_next

            # ---- h0 = ctx @ W_ctx + bctx (split across streams) ----
            for st, (p0, p1, off) in enumerate([(0, 64, 0), (64, 128, CH)]):
                nc.tensor.matmul(
                    pI[p0:p1, :], ones_sb[0:1, :], bctx_sb[0:1, off : off + CH],
                    start=True, stop=False, tile_position=TPS[st],
                    skip_group_check=True,
                )
            for kc in range(NKC):
                wk = wstream.tile([128, D], BF16, tag="wctxk", name=f"wk_{kc}")
                nc.gpsimd.dma_start(out=wk, in_=wctx_t[:, kc, :])
                for st, (p0, p1, off) in enumerate([(0, 64, 0), (64, 128, CH)]):
                    nc.tensor.matmul(
                        pI[p0:p1, :], ctxT_sb[:, kc, :], wk[:, off : off + CH],
                        start=False, stop=(kc == NKC - 1),
                        tile_position=TPS[st], skip_group_check=True,
                    )
            h0_sb = state.tile([BL, D], F32, tag="h", bufs=3)
            h0_bf = state.tile([BL, D], BF16, bufs=3, tag="hbf")
            nc.vector.tensor_copy(h0_sb[:, 0:CH], pI[0:64, :])
            nc.vector.tensor_copy(h0_sb[:, CH:D], pI[64:128, :])
            nc.scalar.copy(h0_bf, h0_sb)
            for j in range(NK):
                transpose_j(h0_bf, j)
            xT0 = make_xT(0)
            nc.vector.tensor_copy(xT0, stage)

            tail_on = mode != "mm_only"
            hprev, hprev_bf, xT_cur, pend = emit_step(
                0, h0_sb, h0_bf, xT0, step1=True, full=True,
                last=(n_steps == 1), pend=None, w_sb=whh_sb,
                bias_sb=bias1_sb, cks=CK_STEP1, tail_on=True,
            )
            if n_steps >= 2:
                hprev, hprev_bf, xT_cur, pend = emit_step(
                    1, hprev, hprev_bf, xT_cur, step1=False, full=True,
                    last=(n_steps == 2), pend=pend, w_sb=wall_sb,
                    bias_sb=biasM_sb, cks=CK_STEADY, tail_on=True,
                )
            for s in range(2, n_steps):
                hprev, hprev_bf, xT_cur, pend = emit_step(
                    s % T, hprev, hprev_bf, xT_cur, step1=False, full=False,
                    last=(s == n_steps - 1), pend=pend, w_sb=wall_sb,
                    bias_sb=biasM_sb, cks=CK_STEADY, tail_on=tail_on,
                )

    nc.finalize()
    return nc


build = _build_nc


def kernel(world_state, goal, W_ctx, b_ctx, start_token, W_ih, b_ih, W_hh, b_hh):
    bf16 = ml_dtypes.bfloat16
    ws = np.asarray(world_state, dtype=np.float32)
    gl = np.asarray(goal, dtype=np.float32)
    W_ctx = np.asarray(W_ctx, dtype=np.float32)
    b_ctx = np.asarray(b_ctx, dtype=np.float32)
    start_token = np.asarray(start_token, dtype=np.float32)
    W_ih = np.asarray(W_ih, dtype=np.float32)
    b_ih = np.asarray(b_ih, dtype=np.float32)
    W_hh = np.asarray(W_hh, dtype=np.float32)
    b_hh = np.asarray(b_hh, dtype=np.float32)

    if "nc" not in _CACHE:
        _CACHE["nc"] = _build_nc()
    nc = _CACHE["nc"]

    ctxT = np.ascontiguousarray(np.concatenate([ws, gl], axis=1).T)
    ctxT_bf = ctxT.astype(bf16)
    wctx_bf = np.ascontiguousarray(W_ctx).astype(bf16)
    whh_bf = np.ascontiguousarray(W_hh).astype(bf16)
    wall_bf = np.ascontiguousarray(
        np.concatenate(
            [W_ih[:, : 2 * D] + W_hh[:, : 2 * D], W_ih[:, 2 * D :], W_hh[:, 2 * D :]],
            axis=1,
        )
    ).astype(bf16)
    gi1 = start_token @ W_ih + b_ih
    bias1 = np.ascontiguousarray(
        np.concatenate([gi1[: 2 * D] + b_hh[: 2 * D], b_hh[2 * D :]])
    ).astype(bf16)[None]
    gin1 = np.ascontiguousarray(gi1[2 * D :].astype(np.float32))[None]
    biasM = np.ascontiguousarray(
        np.concatenate([b_ih[: 2 * D] + b_hh[: 2 * D], b_ih[2 * D :], b_hh[2 * D :]])
    ).astype(bf16)[None]
    bctx = np.ascontiguousarray(b_ctx).astype(bf16)[None]

    shared = dict(
        wctx=wctx_bf, whh=whh_bf, wall=wall_bf, bctx=bctx,
        bias1=bias1, gin1=gin1, biasM=biasM,
    )
    in_maps = [
        {**shared, "ctxT": np.ascontiguousarray(ctxT_bf[:, i * BL : (i + 1) * BL])}
        for i in range(NCORES)
    ]

    LAST_IN_MAPS[0] = in_maps
    res = run_bass_kernel_spmd(
        nc, in_maps, core_ids=list(range(NCORES)), trace=TRACE, **TRACE_KW
    )
    LAST_RESULT[0] = res

    full = np.empty((B, T, D), dtype=np.float32)
    for i in range(NCORES):
        o = np.asarray(res.results[i]["out"])
        full[i * BL : (i + 1) * BL] = o.transpose(1, 0, 2)
    return full
